# revision 3
# baseline (speedup 1.0000x reference)
"""Multi-head attention (B=4, T=2048, D=1024, H=16) on 8 TRN2 NeuronCores.

Sharding: core c = (batch b = c//2, head-group g = c%2). Each core computes
QKV projections for its 8 heads and attention; after a pairwise AllGather of
the per-head attention outputs (per T_q quarter), each core computes the
output projection for its batch restricted to ITS 512 output columns
(columns g*512:(g+1)*512) — no duplicated out-proj work. Host assembles
full[b][:, cols] from cores 2b and 2b+1.

v2 layout highlights (vs the 460us baseline):
  - attn@V runs as full 128-contraction matmuls (PE cost is out-cols *
    pe_cycle regardless of contraction rows): 512 streams instead of 1024,
    one PSUM accumulator per (tq, hp, h2) - the pairwise copy+add in the
    normalize is gone.
  - hp is the OUTER attention loop; V and K/Q projections for later head
    pairs are woven between attention units so ScalarE's exp (the hard
    floor: ~284us of Exp on 33.5M elements, ScalarE is the only engine
    with activation) starts ~17us in instead of ~100us.
  - normalize uses reciprocal_approx_fast (~5x faster than reciprocal).
  - scores are computed transposed (S^T = K_mat @ Q^T per 128-row T_k
    chunk) per (kc, h2) into [128,512] PSUM tiles so exp() lands directly
    in the [T_k, T_q] layout attn@V needs.
  - V carries 64 all-ones columns: attn@V PSUM rows 64..127 are the softmax
    denominator replicated across 64 partitions (free: out-width <= 128
    doesn't change matmul cost), so normalize is partition-aligned.
  - 1/sqrt(d_k) and biases are folded host-side / into PSUM evacuation.
"""

import numpy as np
import ml_dtypes

import concourse.bass as bass
import concourse.tile as tile
from concourse import mybir
from concourse.bass_utils import run_bass_kernel_spmd

BF16 = mybir.dt.bfloat16
F32 = mybir.dt.float32
NPBF16 = ml_dtypes.bfloat16

N_CORES = 8
B, T, D, H = 4, 2048, 1024, 16
DK = D // H          # 64
HL = H // 2          # heads per core (8)
NHP = HL // 2        # head pairs per core (4)
NJP = D // 128       # input-dim 128-chunks (8)
NOC = (D // 2) // 128  # per-core qkv out-dim 128-chunks (4)
NTT = T // 512       # T 512-tiles (4)
NTC = T // 128       # T 128-chunks (16)

_uid = [0]


def _split_sync_commands(nc, max_waits=1, max_updates=1):
    """This walrus build allows only one sync wait/update command on
    sequencer-only (TPB_CTRL) instructions like Drain/NoOp; Tile's kernel
    tail drain carries one wait per logical processor. Split the excess onto
    adjacent same-engine NoOps (program order makes this equivalent)."""
    for func in nc.m.functions:
        for block in func.blocks:
            out = []
            changed = False
            for inst in block.instructions:
                si = inst.sync_info
                if si is None:
                    out.append(inst)
                    continue
                is_dma = "DMA" in type(inst).__name__.upper() or "DMA" in str(
                    getattr(inst, "opcode", "")).upper()
                waits = list(si.on_wait or [])
                # DMA completion increments must stay on the DMA instruction;
                # waits can always move to a preceding same-engine NoOp.
                updates = list(si.on_update or [])
                if is_dma:
                    n_up = len(updates)
                    updates_keep, updates = updates, []
                else:
                    updates_keep = None
                pre, post = [], []
                while len(waits) > max_waits:
                    chunk, waits = waits[:max_waits], waits[max_waits:]
                    _uid[0] += 1
                    pre.append(mybir.InstNoOp(
                        name=f"I-syncsplit-{_uid[0]}", engine=inst.engine,
                        bass_nofuse=True,
                        sync_info=mybir.SyncInfo(on_wait=chunk, on_update=[])))
                while len(updates) > max_updates:
                    chunk, updates = updates[:max_updates], updates[max_updates:]
                    _uid[0] += 1
                    post.append(mybir.InstNoOp(
                        name=f"I-syncsplit-{_uid[0]}", engine=inst.engine,
                        bass_nofuse=True,
                        sync_info=mybir.SyncInfo(on_wait=[], on_update=chunk)))
                if updates_keep is not None:
                    updates = updates_keep
                if pre or post:
                    inst.sync_info = mybir.SyncInfo(on_wait=waits, on_update=updates)
                    changed = True
                out.extend(pre)
                out.append(inst)
                out.extend(post)
            if changed:
                block.instructions = out


def build_nc(split_sync=True):
    nc = bass.Bass("TRN2", target_bir_lowering=False, debug=False,
                   num_devices=N_CORES)

    xt_ext = nc.dram_tensor("xt", [NJP, 128, T], BF16, kind="ExternalInput").ap()
    wq_ext = nc.dram_tensor("wq", [128, NJP, NOC, 128], BF16, kind="ExternalInput").ap()
    wk_ext = nc.dram_tensor("wk", [128, NJP, NOC, 128], BF16, kind="ExternalInput").ap()
    wv_ext = nc.dram_tensor("wv", [128, NJP, 512], BF16, kind="ExternalInput").ap()
    wo_ext = nc.dram_tensor("wo", [128, 2, NHP, 512], BF16, kind="ExternalInput").ap()
    bq_ext = nc.dram_tensor("bq", [128, NOC], F32, kind="ExternalInput").ap()
    bk_ext = nc.dram_tensor("bk", [128, NOC], F32, kind="ExternalInput").ap()
    bv_ext = nc.dram_tensor("bv", [1, 512], F32, kind="ExternalInput").ap()
    bo_ext = nc.dram_tensor("bo", [1, 512], F32, kind="ExternalInput").ap()
    out_ext = nc.dram_tensor("out", [T, 512], F32, kind="ExternalOutput").ap()

    with tile.TileContext(nc) as tc:
        with (
            tc.tile_pool(name="persist", bufs=1) as persist,
            tc.tile_pool(name="epool", bufs=8) as epool,
            tc.tile_pool(name="evac", bufs=2) as evac,
            tc.tile_pool(name="outstage", bufs=2) as outstage,
            tc.tile_pool(name="ps", bufs=3, space="PSUM") as ps_pool,
            tc.tile_pool(name="pv", bufs=3, space="PSUM") as pv_pool,
            tc.tile_pool(name="pp", bufs=2, space="PSUM") as pp_pool,
            tc.tile_pool(name="dram", bufs=1, space="DRAM") as dram,
        ):
            # ---- weights / biases in (K/Q weights first: KQ(oc0) is the
            # critical path to the first exp) ----
            wq_sb = persist.tile([128, NJP, NOC, 128], BF16, tag="wq", name="wq")
            wk_sb = persist.tile([128, NJP, NOC, 128], BF16, tag="wk", name="wk")
            wv_sb = persist.tile([128, NJP, 512], BF16, tag="wv", name="wv")
            wo_sb = persist.tile([128, 2, NHP, 512], BF16, tag="wo", name="wo")
            bq_sb = persist.tile([128, NOC], F32, tag="bq", name="bq")
            bk_sb = persist.tile([128, NOC], F32, tag="bk", name="bk")
            bv_sb = persist.tile([128, 512], F32, tag="bv", name="bv")
            bo_sb = persist.tile([128, 512], F32, tag="bo", name="bo")

            nc.sync.dma_start(out=wk_sb[:], in_=wk_ext[:])
            nc.sync.dma_start(out=wq_sb[:], in_=wq_ext[:])
            xt_sb = []
            for p in range(NJP):
                t_ = persist.tile([128, T], BF16, tag=f"xt{p}", name=f"xt{p}")
                nc.sync.dma_start(out=t_[:], in_=xt_ext[p])
                xt_sb.append(t_)
            nc.sync.dma_start(out=bq_sb[:], in_=bq_ext[:])
            nc.sync.dma_start(out=bk_sb[:], in_=bk_ext[:])
            nc.sync.dma_start(out=wv_sb[:], in_=wv_ext[:])
            # broadcast along partitions (stride-0 partition dim on DRAM side)
            nc.sync.dma_start(
                out=bv_sb[:],
                in_=bass.AP(tensor=bv_ext.tensor, offset=bv_ext.offset,
                            ap=[[0, 128]] + list(bv_ext.ap[1:])))
            nc.sync.dma_start(out=wo_sb[:], in_=wo_ext[:])
            nc.sync.dma_start(
                out=bo_sb[:],
                in_=bass.AP(tensor=bo_ext.tensor, offset=bo_ext.offset,
                            ap=[[0, 128]] + list(bo_ext.ap[1:])))

            qt_sb = [persist.tile([128, T], BF16, tag=f"qt{i}", name=f"qt{i}") for i in range(NOC)]
            kt_sb = [persist.tile([128, T], BF16, tag=f"kt{i}", name=f"kt{i}") for i in range(NOC)]
            vh_sb = [persist.tile([128, HL, 128], BF16, tag=f"vh{i}", name=f"vh{i}") for i in range(NTC)]
            ot_a = [persist.tile([128, NHP, 512], BF16, tag=f"ot_a{q}", name=f"ot_a{q}")
                    for q in range(NTT)]
            ot_b = [persist.tile([128, NHP, 512], BF16, tag=f"ot_b{q}", name=f"ot_b{q}")
                    for q in range(NTT)]

            def emit_kq(oc):
                for w_sb, b_sb, dst in ((wk_sb, bk_sb, kt_sb), (wq_sb, bq_sb, qt_sb)):
                    for tt in range(NTT):
                        psq = pp_pool.tile([128, 512], F32, tag="pp", name="pp")
                        for j in range(NJP):
                            nc.tensor.matmul(
                                psq[:],
                                lhsT=w_sb[:, j, oc, :],
                                rhs=xt_sb[j][:, tt * 512:(tt + 1) * 512],
                                start=(j == 0), stop=(j == NJP - 1))
                        nc.vector.tensor_scalar_add(
                            dst[oc][:, tt * 512:(tt + 1) * 512], psq[:],
                            b_sb[:, oc:oc + 1])

            def emit_v(tcc):
                psv = pp_pool.tile([128, 512], F32, tag="pp", name="pp")
                for j in range(NJP):
                    nc.tensor.matmul(
                        psv[:],
                        lhsT=xt_sb[j][:, tcc * 128:(tcc + 1) * 128],
                        rhs=wv_sb[:, j, :],
                        start=(j == 0), stop=(j == NJP - 1))
                nc.vector.tensor_tensor(
                    vh_sb[tcc][:, :, 0:64],
                    psv[:].rearrange("p (h d) -> p h d", h=HL),
                    bv_sb[:].rearrange("p (h d) -> p h d", h=HL),
                    mybir.AluOpType.add)
                nc.vector.memset(vh_sb[tcc][:, :, 64:128], 1.0)

            def attention(tq, hp, weave=None):
                # weave: optional list of thunks issued between pipeline steps
                # (used to overlap V projection with the first attention unit)
                pv_t = [pv_pool.tile([128, 512], F32, tag="pv", name="pv")
                        for _ in range(2)]
                e_tiles = [[None] * 2 for _ in range(NTC)]

                def emit_scores(kc):
                    for h2 in (0, 1):
                        ps = ps_pool.tile([128, 512], F32, tag="ps", name="ps")
                        nc.tensor.matmul(
                            ps[:],
                            lhsT=kt_sb[hp][h2 * 64:(h2 + 1) * 64, kc * 128:(kc + 1) * 128],
                            rhs=qt_sb[hp][h2 * 64:(h2 + 1) * 64, tq * 512:(tq + 1) * 512],
                            start=True, stop=True,
                            tile_position=(h2 * 64, 0))
                        e_t = epool.tile([128, 512], BF16, tag="e", name="e")
                        nc.scalar.activation(e_t[:], ps[:],
                                             mybir.ActivationFunctionType.Exp)
                        e_tiles[kc][h2] = e_t

                def emit_attnv(kc):
                    for h2 in (0, 1):
                        nc.tensor.matmul(
                            pv_t[h2][:],
                            lhsT=vh_sb[kc][:, 2 * hp + h2, :],
                            rhs=e_tiles[kc][h2][:],
                            start=(kc == 0), stop=(kc == NTC - 1))

                emit_scores(0)
                for kc in range(1, NTC):
                    emit_scores(kc)
                    emit_attnv(kc - 1)
                    if weave and kc - 1 < len(weave):
                        weave[kc - 1]()
                emit_attnv(NTC - 1)
                if weave:
                    for thunk in weave[NTC - 1:]:
                        thunk()
                for h2 in (0, 1):
                    po = pv_t[h2]
                    rr = evac.tile([64, 512], F32, tag="rr", name="rr")
                    nc.vector.reciprocal(rr[:], po[64:128, :])
                    nc.vector.tensor_mul(
                        ot_a[tq][h2 * 64:(h2 + 1) * 64, hp, :],
                        po[0:64, :], rr[:])

            def exchange_and_outproj(tq):
                # pairwise AllGather of this T_q quarter's attention outputs,
                # then this core's 512 out-proj columns for these rows.
                oT_in = dram.tile([128, NHP, 512], BF16, name=f"oT_in{tq}")
                oT_out = dram.tile([2, 128, NHP, 512], BF16, name=f"oT_out{tq}")
                nc.sync.dma_start(out=oT_in[:], in_=ot_a[tq][:])
                nc.gpsimd.collective_compute(
                    "AllGather",
                    mybir.AluOpType.bypass,
                    ins=[oT_in.opt()],
                    outs=[oT_out.opt()],
                    replica_groups=[[0, 1], [2, 3], [4, 5], [6, 7]],
                )
                nc.sync.dma_start(out=ot_a[tq][:], in_=oT_out[0])
                nc.sync.dma_start(out=ot_b[tq][:], in_=oT_out[1])
                for tl in range(4):
                    pso = pp_pool.tile([128, 512], F32, tag="pp", name="pp")
                    first = True
                    for src_i, ot_sb in ((0, ot_a), (1, ot_b)):
                        for hp2 in range(NHP):
                            nc.tensor.matmul(
                                pso[:],
                                lhsT=ot_sb[tq][:, hp2, tl * 128:(tl + 1) * 128],
                                rhs=wo_sb[:, src_i, hp2, :],
                                start=first,
                                stop=(src_i == 1 and hp2 == NHP - 1))
                            first = False
                    ost = outstage.tile([128, 512], F32, tag="ost", name="ost")
                    nc.vector.tensor_add(ost[:], pso[:], bo_sb[:])
                    nc.sync.dma_start(
                        out=out_ext[(tq * 4 + tl) * 128:(tq * 4 + tl + 1) * 128, :],
                        in_=ost[:])

            # ---- schedule ----
            emit_kq(0)
            emit_v(0)
            emit_v(1)
            for hp in range(NHP):
                for tq in range(NTT):
                    if hp == 0 and tq == 0:
                        # weave remaining V projections into the first unit
                        attention(0, 0, weave=[
                            (lambda c=c: emit_v(c)) for c in range(2, NTC)])
                    else:
                        attention(tq, hp)
                    if hp < NHP - 1 and tq == 0:
                        emit_kq(hp + 1)
                    if hp == NHP - 1:
                        exchange_and_outproj(tq)

    if split_sync:
        _split_sync_commands(nc)
    return nc


_NC_CACHE = {}


def _get_nc():
    if "nc" not in _NC_CACHE:
        _NC_CACHE["nc"] = build_nc()
    return _NC_CACHE["nc"]


def _prep_core_inputs(x, Wq, bq, Wk, bk, Wv, bv, Wo, bo):
    """Host-side sharding + layout. Returns in_maps list (8 cores)."""
    x = np.asarray(x, np.float32)
    s = 1.0 / np.sqrt(np.float32(DK))
    Wq_s, bq_s = np.asarray(Wq, np.float32) * s, np.asarray(bq, np.float32) * s
    Wk_f, bk_f = np.asarray(Wk, np.float32), np.asarray(bk, np.float32)
    Wv_f, bv_f = np.asarray(Wv, np.float32), np.asarray(bv, np.float32)
    Wo_f, bo_f = np.asarray(Wo, np.float32), np.asarray(bo, np.float32)

    in_maps = []
    for c in range(N_CORES):
        b, g = c // 2, c % 2
        cols = slice(g * 512, (g + 1) * 512)
        wq_g, bq_g = Wq_s[:, cols], bq_s[cols]
        wk_g, bk_g = Wk_f[:, cols], bk_f[cols]
        wv_g, bv_g = Wv_f[:, cols], bv_f[cols]

        xt_dev = np.ascontiguousarray(x[b].T).reshape(NJP, 128, T).astype(NPBF16)

        def wqk_dev(w):
            # [jp, r, oc, c] -> partitions r, free [jp, oc, c]
            return np.ascontiguousarray(
                w.reshape(NJP, 128, NOC, 128).transpose(1, 0, 2, 3)).astype(NPBF16)

        wv_dev = np.ascontiguousarray(
            wv_g.reshape(NJP, 128, 512).transpose(1, 0, 2)).astype(NPBF16)

        # Wo restricted to this core's 512 output columns, rows regrouped to
        # the on-device O^T layout: [src group, hp, h2, 64] rows ->
        # partitions h2*64+r, free [src, hp, col]
        wo_dev = (Wo_f[:, cols]
                  .reshape(2, NHP, 2, 64, 512)
                  .transpose(2, 3, 0, 1, 4)        # [h2, r, src, hp, col]
                  .reshape(128, 2, NHP, 512)).astype(NPBF16)
        bo_dev = np.ascontiguousarray(bo_f[cols]).reshape(1, 512)

        in_maps.append({
            "xt": xt_dev,
            "wq": wqk_dev(wq_g), "wk": wqk_dev(wk_g), "wv": wv_dev,
            "wo": wo_dev,
            "bq": np.ascontiguousarray(bq_g.reshape(NOC, 128).T),
            "bk": np.ascontiguousarray(bk_g.reshape(NOC, 128).T),
            "bv": bv_g.reshape(1, 512),
            "bo": bo_dev,
        })
    return in_maps


def kernel(x, Wq, bq, Wk, bk, Wv, bv, Wo, bo, _trace=False):
    nc = _get_nc()
    in_maps = _prep_core_inputs(x, Wq, bq, Wk, bk, Wv, bv, Wo, bo)
    res = run_bass_kernel_spmd(nc, in_maps, core_ids=list(range(N_CORES)),
                               trace=_trace)
    out = np.empty((B, T, D), np.float32)
    for b in range(B):
        for g in range(2):
            out[b][:, g * 512:(g + 1) * 512] = res.results[2 * b + g]["out"]
    if _trace:
        kernel.last_result = res
    return out


# revision 4
# speedup vs baseline: 1.1269x; 1.1269x over previous
"""Multi-head attention (B=4, T=2048, D=1024, H=16) on 8 TRN2 NeuronCores.

Sharding: core c = (batch b = c//2, head-group g = c%2). Each core computes
QKV projections for its 8 heads and attention; after a pairwise AllGather of
the per-head attention outputs (per T_q quarter), each core computes the
output projection for its batch restricted to ITS 512 output columns
(columns g*512:(g+1)*512) - no duplicated out-proj work. Host assembles
full[b][:, cols] from cores 2b and 2b+1.

v3 highlights (vs the 460us baseline):
  - EVERY matmul is a uniform (128,128) PE tile - mixing (64,128) and
    (128,128) instructions thrashes the PE tile config (~+240ns + lost
    dual-stream overlap per switch, measured). Scores achieve this with
    zero-padded K tiles: ktz[hp][h2] is [128, T] holding K^T of head h2 in
    its own 64-partition half and ZEROS in the other half, so contracting
    against the full packed Q tile annihilates the other head's rows.
  - attn@V runs as full 128-contraction matmuls (PE cost is out-cols *
    pe_cycle regardless of contraction rows): 512 streams instead of 1024,
    one PSUM accumulator per (tq, hp, h2) - no pairwise copy+add.
  - hp is the OUTER attention loop; V and K/Q projections for later head
    pairs are woven between attention units so ScalarE's exp (the hard
    floor: ~284us of Exp on 33.5M elements; ScalarE is the only engine
    with activation) starts ~17us in instead of ~100us.
  - V carries 64 all-ones columns: attn@V PSUM rows 64..127 are the softmax
    denominator replicated across 64 partitions (free: out-width <= 128
    doesn't change matmul cost), so normalize is partition-aligned.
  - 1/sqrt(d_k) and biases are folded host-side / into PSUM evacuation.
"""

import numpy as np
import ml_dtypes

import concourse.bass as bass
import concourse.tile as tile
from concourse import mybir
from concourse.bass_utils import run_bass_kernel_spmd

BF16 = mybir.dt.bfloat16
F32 = mybir.dt.float32
NPBF16 = ml_dtypes.bfloat16

N_CORES = 8
B, T, D, H = 4, 2048, 1024, 16
DK = D // H          # 64
HL = H // 2          # heads per core (8)
NHP = HL // 2        # head pairs per core (4)
NJP = D // 128       # input-dim 128-chunks (8)
NOC = (D // 2) // 128  # per-core qkv out-dim 128-chunks (4)
NTT = T // 512       # T 512-tiles (4)
NTC = T // 128       # T 128-chunks (16)

_uid = [0]


def _split_sync_commands(nc, max_waits=1, max_updates=1):
    """This walrus build allows only one sync wait/update command on
    sequencer-only (TPB_CTRL) instructions like Drain/NoOp; Tile's kernel
    tail drain carries one wait per logical processor. Split the excess onto
    adjacent same-engine NoOps (program order makes this equivalent)."""
    for func in nc.m.functions:
        for block in func.blocks:
            out = []
            changed = False
            for inst in block.instructions:
                si = inst.sync_info
                if si is None:
                    out.append(inst)
                    continue
                is_dma = "DMA" in type(inst).__name__.upper() or "DMA" in str(
                    getattr(inst, "opcode", "")).upper()
                waits = list(si.on_wait or [])
                # DMA completion increments must stay on the DMA instruction;
                # waits can always move to a preceding same-engine NoOp.
                updates = list(si.on_update or [])
                if is_dma:
                    n_up = len(updates)
                    updates_keep, updates = updates, []
                else:
                    updates_keep = None
                pre, post = [], []
                while len(waits) > max_waits:
                    chunk, waits = waits[:max_waits], waits[max_waits:]
                    _uid[0] += 1
                    pre.append(mybir.InstNoOp(
                        name=f"I-syncsplit-{_uid[0]}", engine=inst.engine,
                        bass_nofuse=True,
                        sync_info=mybir.SyncInfo(on_wait=chunk, on_update=[])))
                while len(updates) > max_updates:
                    chunk, updates = updates[:max_updates], updates[max_updates:]
                    _uid[0] += 1
                    post.append(mybir.InstNoOp(
                        name=f"I-syncsplit-{_uid[0]}", engine=inst.engine,
                        bass_nofuse=True,
                        sync_info=mybir.SyncInfo(on_wait=[], on_update=chunk)))
                if updates_keep is not None:
                    updates = updates_keep
                if pre or post:
                    inst.sync_info = mybir.SyncInfo(on_wait=waits, on_update=updates)
                    changed = True
                out.extend(pre)
                out.append(inst)
                out.extend(post)
            if changed:
                block.instructions = out


def build_nc(split_sync=True):
    nc = bass.Bass("TRN2", target_bir_lowering=False, debug=False,
                   num_devices=N_CORES)

    xt_ext = nc.dram_tensor("xt", [NJP, 128, T], BF16, kind="ExternalInput").ap()
    wq_ext = nc.dram_tensor("wq", [128, NJP, NOC, 128], BF16, kind="ExternalInput").ap()
    wk_ext = nc.dram_tensor("wk", [128, NJP, NOC, 128], BF16, kind="ExternalInput").ap()
    wv_ext = nc.dram_tensor("wv", [128, NJP, 512], BF16, kind="ExternalInput").ap()
    wo_ext = nc.dram_tensor("wo", [128, 2, NHP, 512], BF16, kind="ExternalInput").ap()
    bq_ext = nc.dram_tensor("bq", [128, NOC], F32, kind="ExternalInput").ap()
    bk_ext = nc.dram_tensor("bk", [128, NOC], F32, kind="ExternalInput").ap()
    bv_ext = nc.dram_tensor("bv", [1, 512], F32, kind="ExternalInput").ap()
    bo_ext = nc.dram_tensor("bo", [1, 512], F32, kind="ExternalInput").ap()
    out_ext = nc.dram_tensor("out", [T, 512], F32, kind="ExternalOutput").ap()

    with tile.TileContext(nc) as tc:
        with (
            tc.tile_pool(name="persist", bufs=1) as persist,
            tc.tile_pool(name="epool", bufs=6) as epool,
            tc.tile_pool(name="evac", bufs=2) as evac,
            tc.tile_pool(name="outstage", bufs=2) as outstage,
            tc.tile_pool(name="ps", bufs=2, space="PSUM") as ps_pool,
            tc.tile_pool(name="pv", bufs=2, space="PSUM") as pv_pool,
            tc.tile_pool(name="pp", bufs=2, space="PSUM") as pp_pool,
            tc.tile_pool(name="dram", bufs=1, space="DRAM") as dram,
        ):
            # ---- weights / biases in (K/Q weights first: KQ(oc0) is the
            # critical path to the first exp) ----
            wq_sb = persist.tile([128, NJP, NOC, 128], BF16, tag="wq", name="wq")
            wk_sb = persist.tile([128, NJP, NOC, 128], BF16, tag="wk", name="wk")
            wv_sb = persist.tile([128, NJP, 512], BF16, tag="wv", name="wv")
            wo_sb = persist.tile([128, 2, NHP, 512], BF16, tag="wo", name="wo")
            bq_sb = persist.tile([128, NOC], F32, tag="bq", name="bq")
            bk_sb = persist.tile([128, NOC], F32, tag="bk", name="bk")
            bv_sb = persist.tile([128, 512], F32, tag="bv", name="bv")
            bo_sb = persist.tile([128, 512], F32, tag="bo", name="bo")

            nc.sync.dma_start(out=wk_sb[:], in_=wk_ext[:])
            nc.sync.dma_start(out=wq_sb[:], in_=wq_ext[:])
            xt_sb = []
            for p in range(NJP):
                t_ = persist.tile([128, T], BF16, tag=f"xt{p}", name=f"xt{p}")
                nc.sync.dma_start(out=t_[:], in_=xt_ext[p])
                xt_sb.append(t_)
            nc.sync.dma_start(out=bq_sb[:], in_=bq_ext[:])
            nc.sync.dma_start(out=bk_sb[:], in_=bk_ext[:])
            nc.sync.dma_start(out=wv_sb[:], in_=wv_ext[:])
            # broadcast along partitions (stride-0 partition dim on DRAM side)
            nc.sync.dma_start(
                out=bv_sb[:],
                in_=bass.AP(tensor=bv_ext.tensor, offset=bv_ext.offset,
                            ap=[[0, 128]] + list(bv_ext.ap[1:])))
            nc.sync.dma_start(out=wo_sb[:], in_=wo_ext[:])
            nc.sync.dma_start(
                out=bo_sb[:],
                in_=bass.AP(tensor=bo_ext.tensor, offset=bo_ext.offset,
                            ap=[[0, 128]] + list(bo_ext.ap[1:])))

            qt_sb = [persist.tile([128, T], BF16, tag=f"qt{i}", name=f"qt{i}")
                     for i in range(NOC)]
            # ktz[hp][h2]: [128, T] holding K^T of head 2*hp+h2 in partition
            # rows h2*64..h2*64+63 and ZEROS in the other 64 rows, so scores
            # can contract over the full 128-partition packed Q tile.
            ktz_sb = [[persist.tile([128, T], BF16, tag=f"ktz{i}_{h}",
                                    name=f"ktz{i}_{h}") for h in range(2)]
                      for i in range(NOC)]
            vh_sb = [persist.tile([128, HL, 128], BF16, tag=f"vh{i}", name=f"vh{i}")
                     for i in range(NTC)]
            ot_a = [persist.tile([128, NHP, 512], BF16, tag=f"ot_a{q}", name=f"ot_a{q}")
                    for q in range(NTT)]
            ot_b = [persist.tile([128, NHP, 512], BF16, tag=f"ot_b{q}", name=f"ot_b{q}")
                    for q in range(NTT)]

            # zero the dead halves of ktz once
            for hp in range(NHP):
                nc.vector.memset(ktz_sb[hp][0][64:128, :], 0.0)
                nc.vector.memset(ktz_sb[hp][1][0:64, :], 0.0)

            def emit_kq(oc):
                for tt in range(NTT):
                    psk = pp_pool.tile([128, 512], F32, tag="pp", name="pp")
                    for j in range(NJP):
                        nc.tensor.matmul(
                            psk[:],
                            lhsT=wk_sb[:, j, oc, :],
                            rhs=xt_sb[j][:, tt * 512:(tt + 1) * 512],
                            start=(j == 0), stop=(j == NJP - 1))
                    for h2 in (0, 1):
                        nc.vector.tensor_scalar_add(
                            ktz_sb[oc][h2][h2 * 64:(h2 + 1) * 64,
                                           tt * 512:(tt + 1) * 512],
                            psk[h2 * 64:(h2 + 1) * 64, :],
                            bk_sb[h2 * 64:(h2 + 1) * 64, oc:oc + 1])
                for tt in range(NTT):
                    psq = pp_pool.tile([128, 512], F32, tag="pp", name="pp")
                    for j in range(NJP):
                        nc.tensor.matmul(
                            psq[:],
                            lhsT=wq_sb[:, j, oc, :],
                            rhs=xt_sb[j][:, tt * 512:(tt + 1) * 512],
                            start=(j == 0), stop=(j == NJP - 1))
                    nc.vector.tensor_scalar_add(
                        qt_sb[oc][:, tt * 512:(tt + 1) * 512], psq[:],
                        bq_sb[:, oc:oc + 1])

            def emit_v(tcc):
                psv = pp_pool.tile([128, 512], F32, tag="pp", name="pp")
                for j in range(NJP):
                    nc.tensor.matmul(
                        psv[:],
                        lhsT=xt_sb[j][:, tcc * 128:(tcc + 1) * 128],
                        rhs=wv_sb[:, j, :],
                        start=(j == 0), stop=(j == NJP - 1))
                nc.vector.tensor_tensor(
                    vh_sb[tcc][:, :, 0:64],
                    psv[:].rearrange("p (h d) -> p h d", h=HL),
                    bv_sb[:].rearrange("p (h d) -> p h d", h=HL),
                    mybir.AluOpType.add)
                nc.vector.memset(vh_sb[tcc][:, :, 64:128], 1.0)

            def attention(tq, hp, weave=None):
                # weave: optional list of thunks issued between pipeline steps
                # (used to overlap V/KQ projection with attention units)
                pv_t = [pv_pool.tile([128, 512], F32, tag="pv", name="pv")
                        for _ in range(2)]
                e_tiles = [None] * NTC

                def emit_scores(kc):
                    ps = ps_pool.tile([128, 1024], F32, tag="ps", name="ps")
                    for h2 in (0, 1):
                        nc.tensor.matmul(
                            ps[:, h2 * 512:(h2 + 1) * 512],
                            lhsT=ktz_sb[hp][h2][:, kc * 128:(kc + 1) * 128],
                            rhs=qt_sb[hp][:, tq * 512:(tq + 1) * 512],
                            start=True, stop=True)
                    e_t = epool.tile([128, 1024], BF16, tag="e", name="e")
                    nc.scalar.activation(e_t[:], ps[:],
                                         mybir.ActivationFunctionType.Exp)
                    e_tiles[kc] = e_t

                def emit_attnv(kc):
                    for h2 in (0, 1):
                        nc.tensor.matmul(
                            pv_t[h2][:],
                            lhsT=vh_sb[kc][:, 2 * hp + h2, :],
                            rhs=e_tiles[kc][:, h2 * 512:(h2 + 1) * 512],
                            start=(kc == 0), stop=(kc == NTC - 1))

                emit_scores(0)
                for kc in range(1, NTC):
                    emit_scores(kc)
                    emit_attnv(kc - 1)
                    if weave and kc - 1 < len(weave):
                        weave[kc - 1]()
                emit_attnv(NTC - 1)
                if weave:
                    for thunk in weave[NTC - 1:]:
                        thunk()
                for h2 in (0, 1):
                    po = pv_t[h2]
                    rr = evac.tile([64, 512], F32, tag="rr", name="rr")
                    nc.vector.reciprocal(rr[:], po[64:128, :])
                    nc.vector.tensor_mul(
                        ot_a[tq][h2 * 64:(h2 + 1) * 64, hp, :],
                        po[0:64, :], rr[:])

            def exchange_and_outproj(tq):
                # pairwise AllGather of this T_q quarter's attention outputs,
                # then this core's 512 out-proj columns for these rows.
                oT_in = dram.tile([128, NHP, 512], BF16, name=f"oT_in{tq}")
                oT_out = dram.tile([2, 128, NHP, 512], BF16, name=f"oT_out{tq}")
                nc.sync.dma_start(out=oT_in[:], in_=ot_a[tq][:])
                nc.gpsimd.collective_compute(
                    "AllGather",
                    mybir.AluOpType.bypass,
                    ins=[oT_in.opt()],
                    outs=[oT_out.opt()],
                    replica_groups=[[0, 1], [2, 3], [4, 5], [6, 7]],
                )
                nc.sync.dma_start(out=ot_a[tq][:], in_=oT_out[0])
                nc.sync.dma_start(out=ot_b[tq][:], in_=oT_out[1])
                for tl in range(4):
                    pso = pp_pool.tile([128, 512], F32, tag="pp", name="pp")
                    first = True
                    for src_i, ot_sb in ((0, ot_a), (1, ot_b)):
                        for hp2 in range(NHP):
                            nc.tensor.matmul(
                                pso[:],
                                lhsT=ot_sb[tq][:, hp2, tl * 128:(tl + 1) * 128],
                                rhs=wo_sb[:, src_i, hp2, :],
                                start=first,
                                stop=(src_i == 1 and hp2 == NHP - 1))
                            first = False
                    ost = outstage.tile([128, 512], F32, tag="ost", name="ost")
                    nc.vector.tensor_add(ost[:], pso[:], bo_sb[:])
                    nc.sync.dma_start(
                        out=out_ext[(tq * 4 + tl) * 128:(tq * 4 + tl + 1) * 128, :],
                        in_=ost[:])

            # ---- schedule ----
            emit_kq(0)
            emit_v(0)
            emit_v(1)
            for hp in range(NHP):
                for tq in range(NTT):
                    if hp == 0 and tq == 0:
                        # weave remaining V projections into the first unit
                        attention(0, 0, weave=[
                            (lambda c=c: emit_v(c)) for c in range(2, NTC)])
                    else:
                        attention(tq, hp)
                    if hp < NHP - 1 and tq == 0:
                        emit_kq(hp + 1)
                    if hp == NHP - 1:
                        exchange_and_outproj(tq)

    if split_sync:
        _split_sync_commands(nc)
    return nc


_NC_CACHE = {}


def _get_nc():
    if "nc" not in _NC_CACHE:
        _NC_CACHE["nc"] = build_nc()
    return _NC_CACHE["nc"]


def _prep_core_inputs(x, Wq, bq, Wk, bk, Wv, bv, Wo, bo):
    """Host-side sharding + layout. Returns in_maps list (8 cores)."""
    x = np.asarray(x, np.float32)
    s = 1.0 / np.sqrt(np.float32(DK))
    Wq_s, bq_s = np.asarray(Wq, np.float32) * s, np.asarray(bq, np.float32) * s
    Wk_f, bk_f = np.asarray(Wk, np.float32), np.asarray(bk, np.float32)
    Wv_f, bv_f = np.asarray(Wv, np.float32), np.asarray(bv, np.float32)
    Wo_f, bo_f = np.asarray(Wo, np.float32), np.asarray(bo, np.float32)

    in_maps = []
    for c in range(N_CORES):
        b, g = c // 2, c % 2
        cols = slice(g * 512, (g + 1) * 512)
        wq_g, bq_g = Wq_s[:, cols], bq_s[cols]
        wk_g, bk_g = Wk_f[:, cols], bk_f[cols]
        wv_g, bv_g = Wv_f[:, cols], bv_f[cols]

        xt_dev = np.ascontiguousarray(x[b].T).reshape(NJP, 128, T).astype(NPBF16)

        def wqk_dev(w):
            # [jp, r, oc, c] -> partitions r, free [jp, oc, c]
            return np.ascontiguousarray(
                w.reshape(NJP, 128, NOC, 128).transpose(1, 0, 2, 3)).astype(NPBF16)

        wv_dev = np.ascontiguousarray(
            wv_g.reshape(NJP, 128, 512).transpose(1, 0, 2)).astype(NPBF16)

        # Wo restricted to this core's 512 output columns, rows regrouped to
        # the on-device O^T layout: [src group, hp, h2, 64] rows ->
        # partitions h2*64+r, free [src, hp, col]
        wo_dev = (Wo_f[:, cols]
                  .reshape(2, NHP, 2, 64, 512)
                  .transpose(2, 3, 0, 1, 4)        # [h2, r, src, hp, col]
                  .reshape(128, 2, NHP, 512)).astype(NPBF16)
        bo_dev = np.ascontiguousarray(bo_f[cols]).reshape(1, 512)

        in_maps.append({
            "xt": xt_dev,
            "wq": wqk_dev(wq_g), "wk": wqk_dev(wk_g), "wv": wv_dev,
            "wo": wo_dev,
            "bq": np.ascontiguousarray(bq_g.reshape(NOC, 128).T),
            "bk": np.ascontiguousarray(bk_g.reshape(NOC, 128).T),
            "bv": bv_g.reshape(1, 512),
            "bo": bo_dev,
        })
    return in_maps


def kernel(x, Wq, bq, Wk, bk, Wv, bv, Wo, bo, _trace=False):
    nc = _get_nc()
    in_maps = _prep_core_inputs(x, Wq, bq, Wk, bk, Wv, bv, Wo, bo)
    res = run_bass_kernel_spmd(nc, in_maps, core_ids=list(range(N_CORES)),
                               trace=_trace)
    out = np.empty((B, T, D), np.float32)
    for b in range(B):
        for g in range(2):
            out[b][:, g * 512:(g + 1) * 512] = res.results[2 * b + g]["out"]
    if _trace:
        kernel.last_result = res
    return out


# revision 13
# speedup vs baseline: 1.4061x; 1.2478x over previous
"""Multi-head attention (B=4, T=2048, D=1024, H=16) on 8 TRN2 NeuronCores.

Sharding: core c = (batch b = c//2, head-group g = c%2). Each core computes
QKV projections for its 8 heads and attention; after a pairwise AllGather of
the per-head attention outputs (per T_q quarter), each core computes the
output projection for its batch restricted to ITS 512 output columns
(columns g*512:(g+1)*512) - no duplicated out-proj work. Host assembles
full[b][:, cols] from cores 2b and 2b+1.

v3 highlights (vs the 460us baseline):
  - EVERY matmul is a uniform (128,128) PE tile - mixing (64,128) and
    (128,128) instructions thrashes the PE tile config (~+240ns + lost
    dual-stream overlap per switch, measured). Scores achieve this with
    zero-padded K tiles: ktz[hp][h2] is [128, T] holding K^T of head h2 in
    its own 64-partition half and ZEROS in the other half, so contracting
    against the full packed Q tile annihilates the other head's rows.
  - attn@V runs as full 128-contraction matmuls (PE cost is out-cols *
    pe_cycle regardless of contraction rows): 512 streams instead of 1024,
    one PSUM accumulator per (tq, hp, h2) - no pairwise copy+add.
  - hp is the OUTER attention loop; V and K/Q projections for later head
    pairs are woven between attention units so ScalarE's exp (the hard
    floor: ~284us of Exp on 33.5M elements; ScalarE is the only engine
    with activation) starts ~17us in instead of ~100us.
  - V carries 64 all-ones columns: attn@V PSUM rows 64..127 are the softmax
    denominator replicated across 64 partitions (free: out-width <= 128
    doesn't change matmul cost), so normalize is partition-aligned.
  - 1/sqrt(d_k) and biases are folded host-side / into PSUM evacuation.
"""

import numpy as np
import ml_dtypes

import concourse.bass as bass
import concourse.tile as tile
from concourse import mybir
from concourse.bass_utils import run_bass_kernel_spmd

BF16 = mybir.dt.bfloat16
F32 = mybir.dt.float32
NPBF16 = ml_dtypes.bfloat16

N_CORES = 8
B, T, D, H = 4, 2048, 1024, 16
DK = D // H          # 64
HL = H // 2          # heads per core (8)
NHP = HL // 2        # head pairs per core (4)
NJP = D // 128       # input-dim 128-chunks (8)
NOC = (D // 2) // 128  # per-core qkv out-dim 128-chunks (4)
NTT = T // 512       # T 512-tiles (4)
NTC = T // 128       # T 128-chunks (16)

_uid = [0]


def _split_sync_commands(nc, max_waits=1, max_updates=1):
    """This walrus build allows only one sync wait/update command on
    sequencer-only (TPB_CTRL) instructions like Drain/NoOp; Tile's kernel
    tail drain carries one wait per logical processor. Split the excess onto
    adjacent same-engine NoOps (program order makes this equivalent)."""
    for func in nc.m.functions:
        for block in func.blocks:
            out = []
            changed = False
            for inst in block.instructions:
                si = inst.sync_info
                if si is None:
                    out.append(inst)
                    continue
                is_dma = "DMA" in type(inst).__name__.upper() or "DMA" in str(
                    getattr(inst, "opcode", "")).upper()
                waits = list(si.on_wait or [])
                # DMA completion increments must stay on the DMA instruction;
                # waits can always move to a preceding same-engine NoOp.
                updates = list(si.on_update or [])
                if is_dma:
                    n_up = len(updates)
                    updates_keep, updates = updates, []
                else:
                    updates_keep = None
                pre, post = [], []
                while len(waits) > max_waits:
                    chunk, waits = waits[:max_waits], waits[max_waits:]
                    _uid[0] += 1
                    pre.append(mybir.InstNoOp(
                        name=f"I-syncsplit-{_uid[0]}", engine=inst.engine,
                        bass_nofuse=True,
                        sync_info=mybir.SyncInfo(on_wait=chunk, on_update=[])))
                while len(updates) > max_updates:
                    chunk, updates = updates[:max_updates], updates[max_updates:]
                    _uid[0] += 1
                    post.append(mybir.InstNoOp(
                        name=f"I-syncsplit-{_uid[0]}", engine=inst.engine,
                        bass_nofuse=True,
                        sync_info=mybir.SyncInfo(on_wait=[], on_update=chunk)))
                if updates_keep is not None:
                    updates = updates_keep
                if pre or post:
                    inst.sync_info = mybir.SyncInfo(on_wait=waits, on_update=updates)
                    changed = True
                out.extend(pre)
                out.append(inst)
                out.extend(post)
            if changed:
                block.instructions = out


def build_nc(split_sync=True):
    nc = bass.Bass("TRN2", target_bir_lowering=False, debug=False,
                   num_devices=N_CORES)

    xt_ext = nc.dram_tensor("xt", [NJP, 128, T], BF16, kind="ExternalInput").ap()
    wq_ext = nc.dram_tensor("wq", [128, NJP, NOC, 128], BF16, kind="ExternalInput").ap()
    wk_ext = nc.dram_tensor("wk", [128, NJP, NOC, 128], BF16, kind="ExternalInput").ap()
    wv_ext = nc.dram_tensor("wv", [128, NJP, 512], BF16, kind="ExternalInput").ap()
    wo_ext = nc.dram_tensor("wo", [128, 2, NHP, 512], BF16, kind="ExternalInput").ap()
    bq_ext = nc.dram_tensor("bq", [128, NOC], F32, kind="ExternalInput").ap()
    bk_ext = nc.dram_tensor("bk", [128, NOC], F32, kind="ExternalInput").ap()
    bv_ext = nc.dram_tensor("bv", [1, 512], F32, kind="ExternalInput").ap()
    bo_ext = nc.dram_tensor("bo", [1, 512], F32, kind="ExternalInput").ap()
    out_ext = nc.dram_tensor("out", [T, 512], F32, kind="ExternalOutput").ap()

    with tile.TileContext(nc) as tc:
        with (
            tc.tile_pool(name="persist", bufs=1) as persist,
            tc.tile_pool(name="epool", bufs=6) as epool,
            tc.tile_pool(name="evac", bufs=2) as evac,
            tc.tile_pool(name="outstage", bufs=2) as outstage,
            tc.tile_pool(name="ps", bufs=2, space="PSUM") as ps_pool,
            tc.tile_pool(name="pv", bufs=2, space="PSUM") as pv_pool,
            tc.tile_pool(name="pp", bufs=2, space="PSUM") as pp_pool,
            tc.tile_pool(name="dram", bufs=1, space="DRAM") as dram,
        ):
            # ---- weights / biases in (K/Q weights first: KQ(oc0) is the
            # critical path to the first exp) ----
            wq_sb = persist.tile([128, NJP, NOC, 128], BF16, tag="wq", name="wq")
            wk_sb = persist.tile([128, NJP, NOC, 128], BF16, tag="wk", name="wk")
            wv_sb = persist.tile([128, NJP, 512], BF16, tag="wv", name="wv")
            wo_sb = persist.tile([128, 2, NHP, 512], BF16, tag="wo", name="wo")
            bq_sb = persist.tile([128, NOC], F32, tag="bq", name="bq")
            bk_sb = persist.tile([128, NOC], F32, tag="bk", name="bk")
            bv_sb = persist.tile([128, 512], F32, tag="bv", name="bv")
            bo_sb = persist.tile([128, 512], F32, tag="bo", name="bo")

            nc.sync.dma_start(out=wk_sb[:], in_=wk_ext[:])
            nc.sync.dma_start(out=wq_sb[:], in_=wq_ext[:])
            xt_sb = []
            for p in range(NJP):
                t_ = persist.tile([128, T], BF16, tag=f"xt{p}", name=f"xt{p}")
                nc.sync.dma_start(out=t_[:], in_=xt_ext[p])
                xt_sb.append(t_)
            nc.sync.dma_start(out=bq_sb[:], in_=bq_ext[:])
            nc.sync.dma_start(out=bk_sb[:], in_=bk_ext[:])
            nc.sync.dma_start(out=wv_sb[:], in_=wv_ext[:])
            # broadcast along partitions (stride-0 partition dim on DRAM side)
            nc.sync.dma_start(
                out=bv_sb[:],
                in_=bass.AP(tensor=bv_ext.tensor, offset=bv_ext.offset,
                            ap=[[0, 128]] + list(bv_ext.ap[1:])))
            nc.sync.dma_start(out=wo_sb[:], in_=wo_ext[:])
            nc.sync.dma_start(
                out=bo_sb[:],
                in_=bass.AP(tensor=bo_ext.tensor, offset=bo_ext.offset,
                            ap=[[0, 128]] + list(bo_ext.ap[1:])))

            qt_sb = [persist.tile([128, T], BF16, tag=f"qt{i}", name=f"qt{i}")
                     for i in range(NOC)]
            # ktz[hp][h2]: [128, T] holding K^T of head 2*hp+h2 in partition
            # rows h2*64..h2*64+63 and ZEROS in the other 64 rows, so scores
            # can contract over the full 128-partition packed Q tile.
            ktz_sb = [[persist.tile([128, T], BF16, tag=f"ktz{i}_{h}",
                                    name=f"ktz{i}_{h}") for h in range(2)]
                      for i in range(NOC)]
            vh_sb = [persist.tile([128, HL, 128], BF16, tag=f"vh{i}", name=f"vh{i}")
                     for i in range(NTC)]
            ot_a = [persist.tile([128, NHP, 512], BF16, tag=f"ot_a{q}", name=f"ot_a{q}")
                    for q in range(NTT)]
            ot_b = [persist.tile([128, NHP, 512], BF16, tag=f"ot_b{q}", name=f"ot_b{q}")
                    for q in range(NTT)]

            # zero the dead halves of ktz once
            for hp in range(NHP):
                nc.gpsimd.memset(ktz_sb[hp][0][64:128, :], 0.0)
                nc.gpsimd.memset(ktz_sb[hp][1][0:64, :], 0.0)

            # projection evacuations run on GpSimd (Pool) - keeps them out
            # of the Vector queue so the slow normalize reciprocals can't
            # head-of-line-block them (which stalls the PE via PSUM pool
            # recycling and starves ScalarE's exp).
            def emit_k(oc, tt):
                psk = pp_pool.tile([128, 512], F32, tag="pp", name="pp")
                for j in range(NJP):
                    nc.tensor.matmul(
                        psk[:],
                        lhsT=wk_sb[:, j, oc, :],
                        rhs=xt_sb[j][:, tt * 512:(tt + 1) * 512],
                        start=(j == 0), stop=(j == NJP - 1))
                for h2 in (0, 1):
                    nc.vector.tensor_scalar_add(
                        ktz_sb[oc][h2][h2 * 64:(h2 + 1) * 64,
                                       tt * 512:(tt + 1) * 512],
                        psk[h2 * 64:(h2 + 1) * 64, :],
                        bk_sb[h2 * 64:(h2 + 1) * 64, oc:oc + 1])

            def emit_q(oc, tt):
                psq = pp_pool.tile([128, 512], F32, tag="pp", name="pp")
                for j in range(NJP):
                    nc.tensor.matmul(
                        psq[:],
                        lhsT=wq_sb[:, j, oc, :],
                        rhs=xt_sb[j][:, tt * 512:(tt + 1) * 512],
                        start=(j == 0), stop=(j == NJP - 1))
                nc.vector.tensor_scalar_add(
                    qt_sb[oc][:, tt * 512:(tt + 1) * 512], psq[:],
                    bq_sb[:, oc:oc + 1])

            def emit_kq(oc):
                for tt in range(NTT):
                    emit_k(oc, tt)
                for tt in range(NTT):
                    emit_q(oc, tt)

            def emit_v(tcc):
                psv = pp_pool.tile([128, 512], F32, tag="pp", name="pp")
                for j in range(NJP):
                    nc.tensor.matmul(
                        psv[:],
                        lhsT=xt_sb[j][:, tcc * 128:(tcc + 1) * 128],
                        rhs=wv_sb[:, j, :],
                        start=(j == 0), stop=(j == NJP - 1))
                nc.vector.tensor_tensor(
                    vh_sb[tcc][:, :, 0:64],
                    psv[:].rearrange("p (h d) -> p h d", h=HL),
                    bv_sb[:].rearrange("p (h d) -> p h d", h=HL),
                    mybir.AluOpType.add)
                nc.gpsimd.memset(vh_sb[tcc][:, :, 64:128], 1.0)

            def attention(tq, hp, weave=None):
                # weave: optional list of thunks issued between pipeline steps
                # (used to overlap V/KQ projection with attention units)
                pv_t = [pv_pool.tile([128, 512], F32, tag="pv", name="pv")
                        for _ in range(2)]
                e_tiles = [None] * NTC

                def emit_scores(kc):
                    ps = ps_pool.tile([128, 1024], F32, tag="ps", name="ps")
                    for h2 in (0, 1):
                        nc.tensor.matmul(
                            ps[:, h2 * 512:(h2 + 1) * 512],
                            lhsT=ktz_sb[hp][h2][:, kc * 128:(kc + 1) * 128],
                            rhs=qt_sb[hp][:, tq * 512:(tq + 1) * 512],
                            start=True, stop=True)
                    e_t = epool.tile([128, 1024], BF16, tag="e", name="e")
                    nc.scalar.activation(e_t[:], ps[:],
                                         mybir.ActivationFunctionType.Exp)
                    e_tiles[kc] = e_t

                def emit_attnv(kc):
                    for h2 in (0, 1):
                        nc.tensor.matmul(
                            pv_t[h2][:],
                            lhsT=vh_sb[kc][:, 2 * hp + h2, :],
                            rhs=e_tiles[kc][:, h2 * 512:(h2 + 1) * 512],
                            start=(kc == 0), stop=(kc == NTC - 1))

                emit_scores(0)
                for kc in range(1, NTC):
                    emit_scores(kc)
                    emit_attnv(kc - 1)
                    if weave and kc - 1 < len(weave):
                        weave[kc - 1]()
                emit_attnv(NTC - 1)
                if weave:
                    for thunk in weave[NTC - 1:]:
                        thunk()
                for h2 in (0, 1):
                    po = pv_t[h2]
                    # copy PSUM->SBUF first so the PE's accumulator frees
                    # after ~0.6us instead of being held through the 3.3us
                    # reciprocal (the PE queue is in-order; holding pv here
                    # stalls the next unit's scores and starves exp). The
                    # final multiply runs on GpSimd (SBUF-only) to keep the
                    # vector queue short.
                    sm = evac.tile([128, 512], F32, tag="sm", name="sm")
                    nc.vector.tensor_copy(sm[:], po[:])
                    rr = evac.tile([64, 512], F32, tag="rr", name="rr")
                    nc.vector.reciprocal(rr[:], sm[64:128, :])
                    nc.gpsimd.tensor_mul(
                        ot_a[tq][h2 * 64:(h2 + 1) * 64, hp, :],
                        sm[0:64, :], rr[:])

            def exchange_and_outproj(tq):
                # pairwise AllGather of this T_q quarter's attention outputs,
                # then this core's 512 out-proj columns for these rows.
                oT_in = dram.tile([128, NHP, 512], BF16, name=f"oT_in{tq}")
                oT_out = dram.tile([2, 128, NHP, 512], BF16, name=f"oT_out{tq}")
                nc.sync.dma_start(out=oT_in[:], in_=ot_a[tq][:])
                nc.gpsimd.collective_compute(
                    "AllGather",
                    mybir.AluOpType.bypass,
                    ins=[oT_in.opt()],
                    outs=[oT_out.opt()],
                    replica_groups=[[0, 1], [2, 3], [4, 5], [6, 7]],
                )
                nc.sync.dma_start(out=ot_a[tq][:], in_=oT_out[0])
                nc.sync.dma_start(out=ot_b[tq][:], in_=oT_out[1])
                for tl in range(4):
                    pso = pp_pool.tile([128, 512], F32, tag="pp", name="pp")
                    first = True
                    for src_i, ot_sb in ((0, ot_a), (1, ot_b)):
                        for hp2 in range(NHP):
                            nc.tensor.matmul(
                                pso[:],
                                lhsT=ot_sb[tq][:, hp2, tl * 128:(tl + 1) * 128],
                                rhs=wo_sb[:, src_i, hp2, :],
                                start=first,
                                stop=(src_i == 1 and hp2 == NHP - 1))
                            first = False
                    ost = outstage.tile([128, 512], F32, tag="ost", name="ost")
                    nc.vector.tensor_add(ost[:], pso[:], bo_sb[:])
                    nc.sync.dma_start(
                        out=out_ext[(tq * 4 + tl) * 128:(tq * 4 + tl + 1) * 128, :],
                        in_=ost[:])

            # ---- schedule ----
            # K(oc0) fully + Q(oc0, tt0) are the minimum for the first
            # attention unit (tq=0 reads qt[0][:, 0:512] only); everything
            # else (rest of Q(0), V, later K/Q) weaves into attention units
            # so exp starts as early as possible and the PE never idles.
            for tt in range(NTT):
                emit_k(0, tt)
            emit_q(0, 0)
            emit_v(0)
            emit_v(1)
            for hp in range(NHP):
                for tq in range(NTT):
                    if hp == 0 and tq == 0:
                        # V(c) woven at slot c-2 keeps a 2-slot lead over its
                        # consumer attnV(c) in the in-order PE queue; the
                        # remaining Q(0,tt) evacs (needed by units tq=tt)
                        # trail at the end of the unit.
                        attention(0, 0, weave=(
                            [(lambda c=c: emit_v(c)) for c in range(2, NTC)]
                            + [(lambda t=t: emit_q(0, t)) for t in (1, 2, 3)]))
                    else:
                        attention(tq, hp)
                    if hp < NHP - 1 and tq == 0:
                        emit_kq(hp + 1)
                    if hp == NHP - 1:
                        exchange_and_outproj(tq)

    if split_sync:
        _split_sync_commands(nc)
    return nc


_NC_CACHE = {}


def _get_nc():
    if "nc" not in _NC_CACHE:
        _NC_CACHE["nc"] = build_nc()
    return _NC_CACHE["nc"]


def _prep_core_inputs(x, Wq, bq, Wk, bk, Wv, bv, Wo, bo):
    """Host-side sharding + layout. Returns in_maps list (8 cores)."""
    x = np.asarray(x, np.float32)
    s = 1.0 / np.sqrt(np.float32(DK))
    Wq_s, bq_s = np.asarray(Wq, np.float32) * s, np.asarray(bq, np.float32) * s
    Wk_f, bk_f = np.asarray(Wk, np.float32), np.asarray(bk, np.float32)
    Wv_f, bv_f = np.asarray(Wv, np.float32), np.asarray(bv, np.float32)
    Wo_f, bo_f = np.asarray(Wo, np.float32), np.asarray(bo, np.float32)

    in_maps = []
    for c in range(N_CORES):
        b, g = c // 2, c % 2
        cols = slice(g * 512, (g + 1) * 512)
        wq_g, bq_g = Wq_s[:, cols], bq_s[cols]
        wk_g, bk_g = Wk_f[:, cols], bk_f[cols]
        wv_g, bv_g = Wv_f[:, cols], bv_f[cols]

        xt_dev = np.ascontiguousarray(x[b].T).reshape(NJP, 128, T).astype(NPBF16)

        def wqk_dev(w):
            # [jp, r, oc, c] -> partitions r, free [jp, oc, c]
            return np.ascontiguousarray(
                w.reshape(NJP, 128, NOC, 128).transpose(1, 0, 2, 3)).astype(NPBF16)

        wv_dev = np.ascontiguousarray(
            wv_g.reshape(NJP, 128, 512).transpose(1, 0, 2)).astype(NPBF16)

        # Wo restricted to this core's 512 output columns, rows regrouped to
        # the on-device O^T layout: [src group, hp, h2, 64] rows ->
        # partitions h2*64+r, free [src, hp, col]
        wo_dev = (Wo_f[:, cols]
                  .reshape(2, NHP, 2, 64, 512)
                  .transpose(2, 3, 0, 1, 4)        # [h2, r, src, hp, col]
                  .reshape(128, 2, NHP, 512)).astype(NPBF16)
        bo_dev = np.ascontiguousarray(bo_f[cols]).reshape(1, 512)

        in_maps.append({
            "xt": xt_dev,
            "wq": wqk_dev(wq_g), "wk": wqk_dev(wk_g), "wv": wv_dev,
            "wo": wo_dev,
            "bq": np.ascontiguousarray(bq_g.reshape(NOC, 128).T),
            "bk": np.ascontiguousarray(bk_g.reshape(NOC, 128).T),
            "bv": bv_g.reshape(1, 512),
            "bo": bo_dev,
        })
    return in_maps


def kernel(x, Wq, bq, Wk, bk, Wv, bv, Wo, bo, _trace=False):
    nc = _get_nc()
    in_maps = _prep_core_inputs(x, Wq, bq, Wk, bk, Wv, bv, Wo, bo)
    res = run_bass_kernel_spmd(nc, in_maps, core_ids=list(range(N_CORES)),
                               trace=_trace)
    out = np.empty((B, T, D), np.float32)
    for b in range(B):
        for g in range(2):
            out[b][:, g * 512:(g + 1) * 512] = res.results[2 * b + g]["out"]
    if _trace:
        kernel.last_result = res
    return out


# revision 15
# speedup vs baseline: 1.4713x; 1.0464x over previous
"""Multi-head attention (B=4, T=2048, D=1024, H=16) on 8 TRN2 NeuronCores.

Sharding: core c = (batch b = c//2, head-group g = c%2). Each core computes
QKV projections for its 8 heads and attention; after a pairwise AllGather of
the per-head attention outputs (per T_q quarter), each core computes the
output projection for its batch restricted to ITS 512 output columns
(columns g*512:(g+1)*512) - no duplicated out-proj work. Host assembles
full[b][:, cols] from cores 2b and 2b+1.

v3 highlights (vs the 460us baseline):
  - EVERY matmul is a uniform (128,128) PE tile - mixing (64,128) and
    (128,128) instructions thrashes the PE tile config (~+240ns + lost
    dual-stream overlap per switch, measured). Scores achieve this with
    zero-padded K tiles: ktz[hp][h2] is [128, T] holding K^T of head h2 in
    its own 64-partition half and ZEROS in the other half, so contracting
    against the full packed Q tile annihilates the other head's rows.
  - attn@V runs as full 128-contraction matmuls (PE cost is out-cols *
    pe_cycle regardless of contraction rows): 512 streams instead of 1024,
    one PSUM accumulator per (tq, hp, h2) - no pairwise copy+add.
  - hp is the OUTER attention loop; V and K/Q projections for later head
    pairs are woven between attention units so ScalarE's exp (the hard
    floor: ~284us of Exp on 33.5M elements; ScalarE is the only engine
    with activation) starts ~17us in instead of ~100us.
  - V carries 64 all-ones columns: attn@V PSUM rows 64..127 are the softmax
    denominator replicated across 64 partitions (free: out-width <= 128
    doesn't change matmul cost), so normalize is partition-aligned.
  - 1/sqrt(d_k) and biases are folded host-side / into PSUM evacuation.
"""

import numpy as np
import ml_dtypes

import concourse.bass as bass
import concourse.tile as tile
from concourse import mybir
from concourse.bass_utils import run_bass_kernel_spmd

BF16 = mybir.dt.bfloat16
F32 = mybir.dt.float32
NPBF16 = ml_dtypes.bfloat16

N_CORES = 8
B, T, D, H = 4, 2048, 1024, 16
DK = D // H          # 64
HL = H // 2          # heads per core (8)
NHP = HL // 2        # head pairs per core (4)
NJP = D // 128       # input-dim 128-chunks (8)
NOC = (D // 2) // 128  # per-core qkv out-dim 128-chunks (4)
NTT = T // 512       # T 512-tiles (4)
NTC = T // 128       # T 128-chunks (16)

_uid = [0]


def _split_sync_commands(nc, max_waits=1, max_updates=1):
    """This walrus build allows only one sync wait/update command on
    sequencer-only (TPB_CTRL) instructions like Drain/NoOp; Tile's kernel
    tail drain carries one wait per logical processor. Split the excess onto
    adjacent same-engine NoOps (program order makes this equivalent)."""
    for func in nc.m.functions:
        for block in func.blocks:
            out = []
            changed = False
            for inst in block.instructions:
                si = inst.sync_info
                if si is None:
                    out.append(inst)
                    continue
                is_dma = "DMA" in type(inst).__name__.upper() or "DMA" in str(
                    getattr(inst, "opcode", "")).upper()
                waits = list(si.on_wait or [])
                # DMA completion increments must stay on the DMA instruction;
                # waits can always move to a preceding same-engine NoOp.
                updates = list(si.on_update or [])
                if is_dma:
                    n_up = len(updates)
                    updates_keep, updates = updates, []
                else:
                    updates_keep = None
                pre, post = [], []
                while len(waits) > max_waits:
                    chunk, waits = waits[:max_waits], waits[max_waits:]
                    _uid[0] += 1
                    pre.append(mybir.InstNoOp(
                        name=f"I-syncsplit-{_uid[0]}", engine=inst.engine,
                        bass_nofuse=True,
                        sync_info=mybir.SyncInfo(on_wait=chunk, on_update=[])))
                while len(updates) > max_updates:
                    chunk, updates = updates[:max_updates], updates[max_updates:]
                    _uid[0] += 1
                    post.append(mybir.InstNoOp(
                        name=f"I-syncsplit-{_uid[0]}", engine=inst.engine,
                        bass_nofuse=True,
                        sync_info=mybir.SyncInfo(on_wait=[], on_update=chunk)))
                if updates_keep is not None:
                    updates = updates_keep
                if pre or post:
                    inst.sync_info = mybir.SyncInfo(on_wait=waits, on_update=updates)
                    changed = True
                out.extend(pre)
                out.append(inst)
                out.extend(post)
            if changed:
                block.instructions = out


def build_nc(split_sync=True):
    nc = bass.Bass("TRN2", target_bir_lowering=False, debug=False,
                   num_devices=N_CORES)

    xt_ext = nc.dram_tensor("xt", [NJP, 128, T], BF16, kind="ExternalInput").ap()
    wq_ext = nc.dram_tensor("wq", [NOC, 128, NJP, 128], BF16, kind="ExternalInput").ap()
    wk_ext = nc.dram_tensor("wk", [NOC, 128, NJP, 128], BF16, kind="ExternalInput").ap()
    wv_ext = nc.dram_tensor("wv", [128, NJP, 512], BF16, kind="ExternalInput").ap()
    wo_ext = nc.dram_tensor("wo", [128, 2, NHP, 512], BF16, kind="ExternalInput").ap()
    bq_ext = nc.dram_tensor("bq", [128, NOC], F32, kind="ExternalInput").ap()
    bk_ext = nc.dram_tensor("bk", [128, NOC], F32, kind="ExternalInput").ap()
    bv_ext = nc.dram_tensor("bv", [1, 512], F32, kind="ExternalInput").ap()
    bo_ext = nc.dram_tensor("bo", [1, 512], F32, kind="ExternalInput").ap()
    out_ext = nc.dram_tensor("out", [T, 512], F32, kind="ExternalOutput").ap()

    with tile.TileContext(nc) as tc:
        with (
            tc.tile_pool(name="persist", bufs=1) as persist,
            tc.tile_pool(name="epool", bufs=6) as epool,
            tc.tile_pool(name="evac", bufs=2) as evac,
            tc.tile_pool(name="outstage", bufs=2) as outstage,
            tc.tile_pool(name="ps", bufs=2, space="PSUM") as ps_pool,
            tc.tile_pool(name="pv", bufs=2, space="PSUM") as pv_pool,
            tc.tile_pool(name="pp", bufs=2, space="PSUM") as pp_pool,
            tc.tile_pool(name="dram", bufs=1, space="DRAM") as dram,
        ):
            # ---- weights / biases in (K/Q weights first: KQ(oc0) is the
            # critical path to the first exp) ----
            wq_sb = [persist.tile([128, NJP, 128], BF16, tag=f"wq{o}", name=f"wq{o}")
                     for o in range(NOC)]
            wk_sb = [persist.tile([128, NJP, 128], BF16, tag=f"wk{o}", name=f"wk{o}")
                     for o in range(NOC)]
            wv_sb = persist.tile([128, NJP, 512], BF16, tag="wv", name="wv")
            wo_sb = persist.tile([128, 2, NHP, 512], BF16, tag="wo", name="wo")
            bq_sb = persist.tile([128, NOC], F32, tag="bq", name="bq")
            bk_sb = persist.tile([128, NOC], F32, tag="bk", name="bk")
            bv_sb = persist.tile([128, 512], F32, tag="bv", name="bv")
            bo_sb = persist.tile([128, 512], F32, tag="bo", name="bo")

            # oc=0 weights first - they gate the first scores/exp; the rest
            # of the weights trail behind the x chunks.
            nc.sync.dma_start(out=wk_sb[0][:], in_=wk_ext[0])
            nc.sync.dma_start(out=wq_sb[0][:], in_=wq_ext[0])
            xt_sb = []
            for p in range(NJP):
                t_ = persist.tile([128, T], BF16, tag=f"xt{p}", name=f"xt{p}")
                nc.sync.dma_start(out=t_[:], in_=xt_ext[p])
                xt_sb.append(t_)
            nc.sync.dma_start(out=bq_sb[:], in_=bq_ext[:])
            nc.sync.dma_start(out=bk_sb[:], in_=bk_ext[:])
            nc.sync.dma_start(out=wv_sb[:], in_=wv_ext[:])
            for o in range(1, NOC):
                nc.sync.dma_start(out=wk_sb[o][:], in_=wk_ext[o])
                nc.sync.dma_start(out=wq_sb[o][:], in_=wq_ext[o])
            # broadcast along partitions (stride-0 partition dim on DRAM side)
            nc.sync.dma_start(
                out=bv_sb[:],
                in_=bass.AP(tensor=bv_ext.tensor, offset=bv_ext.offset,
                            ap=[[0, 128]] + list(bv_ext.ap[1:])))
            nc.sync.dma_start(out=wo_sb[:], in_=wo_ext[:])
            nc.sync.dma_start(
                out=bo_sb[:],
                in_=bass.AP(tensor=bo_ext.tensor, offset=bo_ext.offset,
                            ap=[[0, 128]] + list(bo_ext.ap[1:])))

            qt_sb = [persist.tile([128, T], BF16, tag=f"qt{i}", name=f"qt{i}")
                     for i in range(NOC)]
            # ktz[hp][h2]: [128, T] holding K^T of head 2*hp+h2 in partition
            # rows h2*64..h2*64+63 and ZEROS in the other 64 rows, so scores
            # can contract over the full 128-partition packed Q tile.
            ktz_sb = [[persist.tile([128, T], BF16, tag=f"ktz{i}_{h}",
                                    name=f"ktz{i}_{h}") for h in range(2)]
                      for i in range(NOC)]
            vh_sb = [persist.tile([128, HL, 128], BF16, tag=f"vh{i}", name=f"vh{i}")
                     for i in range(NTC)]
            ot_a = [persist.tile([128, NHP, 512], BF16, tag=f"ot_a{q}", name=f"ot_a{q}")
                    for q in range(NTT)]
            ot_b = [persist.tile([128, NHP, 512], BF16, tag=f"ot_b{q}", name=f"ot_b{q}")
                    for q in range(NTT)]

            # zero the dead halves of ktz and pre-fill the all-ones V
            # columns once, on GpSimd, during the input DMA window
            for hp in range(NHP):
                nc.gpsimd.memset(ktz_sb[hp][0][64:128, :], 0.0)
                nc.gpsimd.memset(ktz_sb[hp][1][0:64, :], 0.0)
            for tcc in range(NTC):
                nc.gpsimd.memset(vh_sb[tcc][:, :, 64:128], 1.0)

            # projection evacuations run on GpSimd (Pool) - keeps them out
            # of the Vector queue so the slow normalize reciprocals can't
            # head-of-line-block them (which stalls the PE via PSUM pool
            # recycling and starves ScalarE's exp).
            def emit_k(oc, tt):
                psk = pp_pool.tile([128, 512], F32, tag="pp", name="pp")
                for j in range(NJP):
                    nc.tensor.matmul(
                        psk[:],
                        lhsT=wk_sb[oc][:, j, :],
                        rhs=xt_sb[j][:, tt * 512:(tt + 1) * 512],
                        start=(j == 0), stop=(j == NJP - 1))
                for h2 in (0, 1):
                    nc.vector.tensor_scalar_add(
                        ktz_sb[oc][h2][h2 * 64:(h2 + 1) * 64,
                                       tt * 512:(tt + 1) * 512],
                        psk[h2 * 64:(h2 + 1) * 64, :],
                        bk_sb[h2 * 64:(h2 + 1) * 64, oc:oc + 1])

            def emit_q(oc, tt):
                psq = pp_pool.tile([128, 512], F32, tag="pp", name="pp")
                for j in range(NJP):
                    nc.tensor.matmul(
                        psq[:],
                        lhsT=wq_sb[oc][:, j, :],
                        rhs=xt_sb[j][:, tt * 512:(tt + 1) * 512],
                        start=(j == 0), stop=(j == NJP - 1))
                nc.vector.tensor_scalar_add(
                    qt_sb[oc][:, tt * 512:(tt + 1) * 512], psq[:],
                    bq_sb[:, oc:oc + 1])

            def emit_kq(oc):
                for tt in range(NTT):
                    emit_k(oc, tt)
                for tt in range(NTT):
                    emit_q(oc, tt)

            def emit_v(tcc):
                psv = pp_pool.tile([128, 512], F32, tag="pp", name="pp")
                for j in range(NJP):
                    nc.tensor.matmul(
                        psv[:],
                        lhsT=xt_sb[j][:, tcc * 128:(tcc + 1) * 128],
                        rhs=wv_sb[:, j, :],
                        start=(j == 0), stop=(j == NJP - 1))
                nc.vector.tensor_tensor(
                    vh_sb[tcc][:, :, 0:64],
                    psv[:].rearrange("p (h d) -> p h d", h=HL),
                    bv_sb[:].rearrange("p (h d) -> p h d", h=HL),
                    mybir.AluOpType.add)

            def attention(tq, hp, weave=None):
                # weave: optional list of thunks issued between pipeline steps
                # (used to overlap V/KQ projection with attention units)
                pv_t = [pv_pool.tile([128, 512], F32, tag="pv", name="pv")
                        for _ in range(2)]
                e_tiles = [None] * NTC

                def emit_scores(kc):
                    ps = ps_pool.tile([128, 1024], F32, tag="ps", name="ps")
                    for h2 in (0, 1):
                        nc.tensor.matmul(
                            ps[:, h2 * 512:(h2 + 1) * 512],
                            lhsT=ktz_sb[hp][h2][:, kc * 128:(kc + 1) * 128],
                            rhs=qt_sb[hp][:, tq * 512:(tq + 1) * 512],
                            start=True, stop=True)
                    e_t = epool.tile([128, 1024], BF16, tag="e", name="e")
                    nc.scalar.activation(e_t[:], ps[:],
                                         mybir.ActivationFunctionType.Exp)
                    e_tiles[kc] = e_t

                def emit_attnv(kc):
                    for h2 in (0, 1):
                        nc.tensor.matmul(
                            pv_t[h2][:],
                            lhsT=vh_sb[kc][:, 2 * hp + h2, :],
                            rhs=e_tiles[kc][:, h2 * 512:(h2 + 1) * 512],
                            start=(kc == 0), stop=(kc == NTC - 1))

                emit_scores(0)
                for kc in range(1, NTC):
                    emit_scores(kc)
                    emit_attnv(kc - 1)
                    if weave and kc - 1 < len(weave) and weave[kc - 1]:
                        weave[kc - 1]()
                emit_attnv(NTC - 1)
                if weave:
                    for thunk in weave[NTC - 1:]:
                        if thunk:
                            thunk()
                for h2 in (0, 1):
                    po = pv_t[h2]
                    # copy PSUM->SBUF first so the PE's accumulator frees
                    # after ~0.6us instead of being held through the 3.3us
                    # reciprocal (the PE queue is in-order; holding pv here
                    # stalls the next unit's scores and starves exp). The
                    # final multiply runs on GpSimd (SBUF-only) to keep the
                    # vector queue short.
                    sm = evac.tile([128, 512], F32, tag="sm", name="sm")
                    nc.vector.tensor_copy(sm[:], po[:])
                    rr = evac.tile([64, 512], F32, tag="rr", name="rr")
                    nc.vector.reciprocal(rr[:], sm[64:128, :])
                    nc.gpsimd.tensor_mul(
                        ot_a[tq][h2 * 64:(h2 + 1) * 64, hp, :],
                        sm[0:64, :], rr[:])

            def exchange_and_outproj(tq):
                # pairwise AllGather of this T_q quarter's attention outputs,
                # then this core's 512 out-proj columns for these rows.
                oT_in = dram.tile([128, NHP, 512], BF16, name=f"oT_in{tq}")
                oT_out = dram.tile([2, 128, NHP, 512], BF16, name=f"oT_out{tq}")
                nc.sync.dma_start(out=oT_in[:], in_=ot_a[tq][:])
                nc.gpsimd.collective_compute(
                    "AllGather",
                    mybir.AluOpType.bypass,
                    ins=[oT_in.opt()],
                    outs=[oT_out.opt()],
                    replica_groups=[[0, 1], [2, 3], [4, 5], [6, 7]],
                )
                nc.sync.dma_start(out=ot_a[tq][:], in_=oT_out[0])
                nc.sync.dma_start(out=ot_b[tq][:], in_=oT_out[1])
                for tl in range(4):
                    pso = pp_pool.tile([128, 512], F32, tag="pp", name="pp")
                    first = True
                    for src_i, ot_sb in ((0, ot_a), (1, ot_b)):
                        for hp2 in range(NHP):
                            nc.tensor.matmul(
                                pso[:],
                                lhsT=ot_sb[tq][:, hp2, tl * 128:(tl + 1) * 128],
                                rhs=wo_sb[:, src_i, hp2, :],
                                start=first,
                                stop=(src_i == 1 and hp2 == NHP - 1))
                            first = False
                    ost = outstage.tile([128, 512], F32, tag="ost", name="ost")
                    nc.vector.tensor_add(ost[:], pso[:], bo_sb[:])
                    nc.sync.dma_start(
                        out=out_ext[(tq * 4 + tl) * 128:(tq * 4 + tl + 1) * 128, :],
                        in_=ost[:])

            # ---- schedule ----
            # K(oc0) fully + Q(oc0, tt0) are the minimum for the first
            # attention unit (tq=0 reads qt[0][:, 0:512] only). Everything
            # else (rest of Q(0), V, later K/Q) weaves INSIDE attention
            # units at spread slots, so scores production (and with it
            # ScalarE's exp) is never interrupted by a projection block.
            for tt in range(NTT):
                emit_k(0, tt)
            emit_q(0, 0)
            emit_v(0)
            emit_v(1)

            def kq_chains(oc):
                return ([(lambda t=t, o=oc: emit_k(o, t)) for t in range(NTT)]
                        + [(lambda t=t, o=oc: emit_q(o, t)) for t in range(NTT)])

            for hp in range(NHP):
                # 8 projection chains for the next head pair, spread 3/3/2
                # across units (1..3, hp) at slots {2, 7, 12}
                nxt = kq_chains(hp + 1) if hp < NHP - 1 else []
                for tq in range(NTT):
                    if hp == 0 and tq == 0:
                        # V(c) woven at slot c-2 keeps a 2-slot lead over its
                        # consumer attnV(c) in the in-order PE queue; the
                        # remaining Q(0,tt) evacs (needed by units tq=tt)
                        # trail at the end of the unit.
                        attention(0, 0, weave=(
                            [(lambda c=c: emit_v(c)) for c in range(2, NTC)]
                            + [(lambda t=t: emit_q(0, t)) for t in (1, 2, 3)]))
                    elif tq >= 1 and nxt:
                        batch, nxt = nxt[:3], nxt[3:]
                        w = [None] * NTC
                        for i, ch in enumerate(batch):
                            w[(2, 7, 12)[i]] = ch
                        attention(tq, hp, weave=w)
                    else:
                        attention(tq, hp)
                    if hp == NHP - 1:
                        exchange_and_outproj(tq)

    if split_sync:
        _split_sync_commands(nc)
    return nc


_NC_CACHE = {}


def _get_nc():
    if "nc" not in _NC_CACHE:
        _NC_CACHE["nc"] = build_nc()
    return _NC_CACHE["nc"]


def _prep_core_inputs(x, Wq, bq, Wk, bk, Wv, bv, Wo, bo):
    """Host-side sharding + layout. Returns in_maps list (8 cores)."""
    x = np.asarray(x, np.float32)
    s = 1.0 / np.sqrt(np.float32(DK))
    Wq_s, bq_s = np.asarray(Wq, np.float32) * s, np.asarray(bq, np.float32) * s
    Wk_f, bk_f = np.asarray(Wk, np.float32), np.asarray(bk, np.float32)
    Wv_f, bv_f = np.asarray(Wv, np.float32), np.asarray(bv, np.float32)
    Wo_f, bo_f = np.asarray(Wo, np.float32), np.asarray(bo, np.float32)

    in_maps = []
    for c in range(N_CORES):
        b, g = c // 2, c % 2
        cols = slice(g * 512, (g + 1) * 512)
        wq_g, bq_g = Wq_s[:, cols], bq_s[cols]
        wk_g, bk_g = Wk_f[:, cols], bk_f[cols]
        wv_g, bv_g = Wv_f[:, cols], bv_f[cols]

        xt_dev = np.ascontiguousarray(x[b].T).reshape(NJP, 128, T).astype(NPBF16)

        def wqk_dev(w):
            # [jp, r, oc, c] -> [oc, partitions r, jp, c] (one DMA per oc)
            return np.ascontiguousarray(
                w.reshape(NJP, 128, NOC, 128).transpose(2, 1, 0, 3)).astype(NPBF16)

        wv_dev = np.ascontiguousarray(
            wv_g.reshape(NJP, 128, 512).transpose(1, 0, 2)).astype(NPBF16)

        # Wo restricted to this core's 512 output columns, rows regrouped to
        # the on-device O^T layout: [src group, hp, h2, 64] rows ->
        # partitions h2*64+r, free [src, hp, col]
        wo_dev = (Wo_f[:, cols]
                  .reshape(2, NHP, 2, 64, 512)
                  .transpose(2, 3, 0, 1, 4)        # [h2, r, src, hp, col]
                  .reshape(128, 2, NHP, 512)).astype(NPBF16)
        bo_dev = np.ascontiguousarray(bo_f[cols]).reshape(1, 512)

        in_maps.append({
            "xt": xt_dev,
            "wq": wqk_dev(wq_g), "wk": wqk_dev(wk_g), "wv": wv_dev,
            "wo": wo_dev,
            "bq": np.ascontiguousarray(bq_g.reshape(NOC, 128).T),
            "bk": np.ascontiguousarray(bk_g.reshape(NOC, 128).T),
            "bv": bv_g.reshape(1, 512),
            "bo": bo_dev,
        })
    return in_maps


def kernel(x, Wq, bq, Wk, bk, Wv, bv, Wo, bo, _trace=False):
    nc = _get_nc()
    in_maps = _prep_core_inputs(x, Wq, bq, Wk, bk, Wv, bv, Wo, bo)
    res = run_bass_kernel_spmd(nc, in_maps, core_ids=list(range(N_CORES)),
                               trace=_trace)
    out = np.empty((B, T, D), np.float32)
    for b in range(B):
        for g in range(2):
            out[b][:, g * 512:(g + 1) * 512] = res.results[2 * b + g]["out"]
    if _trace:
        kernel.last_result = res
    return out
